# revision 6
# baseline (speedup 1.0000x reference)
"""2-layer HGT (patient/drug) kernel for 8x TRN2 NeuronCores via Bass/Tile.

Strategy:
  - Host: fuse per-head relation matrices R_k/R_v into the K/V projection
    weights, fold p_rel*scale into Q weights, fold sigmoid(skip) into the
    output projection. Permute nodes so destination nodes are degree-sorted
    and dealt round-robin across cores (balanced edges AND nodes); each core
    owns a contiguous shard of permuted node ids.
  - Device (SPMD, one program): feature-major dense projections (TensorE),
    per-edge gather of fused KV rows + Q rows (indirect DMA), logits by
    elementwise mul + segmented reduce (DVE), exp (ACT, no max-subtraction
    needed: softmax is shift-invariant and logits are O(1) here), and
    segment-sum via one-hot selection matmuls accumulating in PSUM.
    One AllGather of node features between the two layers.
"""

import numpy as np

import concourse.bacc as bacc
import concourse.bass as bass
import concourse.mybir as mybir
import concourse.tile as tile
from concourse import bass_utils
from concourse.masks import make_identity

FP32 = mybir.dt.float32
I32 = mybir.dt.int32

# ---- problem constants (hardcoded per contract) ----
N_P, N_D = 50000, 5000
IN_P, IN_D = 128, 64
HID, H, D = 256, 4, 64
NCORES = 8
P = 128
SH_P = 6272          # patient shard cols per core (49 tiles of 128)
SH_D = 640           # drug shard cols per core (5 tiles)
NP_PAD = SH_P * NCORES   # 50176
ND_PAD = SH_D * NCORES   # 5120
NT_P = SH_P // P     # 49
NT_D = SH_D // P     # 5
EW = 4               # max 128-edge subgroups per iteration
SH_ALL = SH_P + SH_D  # 6912

KV_W = 2 * HID       # 512 fused (k_rel | v_rel) row width


# ================= host-side preprocessing =================

def _fuse_weights(params):
    """Returns dict of numpy arrays for device consumption."""
    out = {}
    scale = np.float32(1.0 / np.sqrt(D))
    # input projections
    for t, key in (("p", "patient"), ("d", "drug")):
        W = np.asarray(params["lin"][key]["W"], np.float32)
        b = np.asarray(params["lin"][key]["b"], np.float32)
        out[f"wlin_{t}"] = np.ascontiguousarray(W)
        out[f"blin_{t}"] = np.ascontiguousarray(b.reshape(2, 128).T)  # [128,2] col=fo chunk
    et_src = {"p": "patient__takes__drug", "d": "drug__rev_takes__patient"}
    et_dst = {"p": "drug__rev_takes__patient", "d": "patient__takes__drug"}
    for L, lp in enumerate(params["layers"]):
        for t, key in (("p", "patient"), ("d", "drug")):
            W = np.asarray(lp["kqv"][key]["W"], np.float32)   # [256, 768]
            b = np.asarray(lp["kqv"][key]["b"], np.float32)   # [768]
            Wk, Wq, Wv = W[:, :HID], W[:, HID:2 * HID], W[:, 2 * HID:]
            bk, bq, bv = b[:HID], b[HID:2 * HID], b[2 * HID:]
            Rk = np.asarray(lp["k_rel"][et_src[t]], np.float32)  # [H, D, D]
            Rv = np.asarray(lp["v_rel"][et_src[t]], np.float32)
            prel = np.asarray(lp["p_rel"][et_dst[t]], np.float32)  # [H]
            # k_rel fused (bias dropped: per-dst softmax shift invariance)
            Wk4 = Wk.reshape(HID, H, D)
            Wkf = np.einsum("fhd,hde->fhe", Wk4, Rk).reshape(HID, HID)
            Wv4 = Wv.reshape(HID, H, D)
            Wvf = np.einsum("fhd,hde->fhe", Wv4, Rv).reshape(HID, HID)
            bvf = np.einsum("hd,hde->he", bv.reshape(H, D), Rv).reshape(HID)
            # q scaled by p_rel*scale per head
            qs = (prel * scale).repeat(D)  # [256]
            Wqf = Wq * qs[None, :]
            bqf = bq * qs
            sk = 1.0 / (1.0 + np.exp(-np.float64(lp["skip"][key])))
            sk = np.float32(sk)
            Wo = np.asarray(lp["out"][key]["W"], np.float32) * sk
            bo = np.asarray(lp["out"][key]["b"], np.float32) * sk
            wkv = np.concatenate([Wkf, Wvf], axis=1)  # [256,512]
            out[f"wkv_{t}_{L}"] = np.ascontiguousarray(
                wkv.reshape(2, 128, KV_W).transpose(1, 0, 2).reshape(128, 2 * KV_W))
            out[f"wq_{t}_{L}"] = np.ascontiguousarray(
                Wqf.reshape(2, 128, HID).transpose(1, 0, 2).reshape(128, 2 * HID))
            out[f"bq_{t}_{L}"] = np.ascontiguousarray(bqf.reshape(1, HID))  # [1,256]
            out[f"wout_{t}_{L}"] = np.ascontiguousarray(
                Wo.reshape(2, 128, HID).transpose(1, 0, 2).reshape(128, 2 * HID))
            out[f"bout_{t}_{L}"] = np.ascontiguousarray(bo.reshape(2, 128).T)  # [128,2]
            out[f"bv_{t}_{L}"] = np.ascontiguousarray(bvf.reshape(2, 128).T)   # [128,2]
            out[f"skipc_{t}_{L}"] = np.float32(1.0 - sk)  # python-side const
    return out


def _shard_perm(deg, n, shard):
    """Degree-sorted round-robin dealing. Returns new_of_old [n] -> new id."""
    order = np.argsort(-deg, kind="stable")
    idx = np.arange(n, dtype=np.int64)
    new_ids = (idx % NCORES) * shard + idx // NCORES
    new_of_old = np.empty(n, dtype=np.int64)
    new_of_old[order] = new_ids
    return new_of_old


def _plan_and_fill(src_old, dst_old, new_src, new_dst, shard_dst, ntiles):
    """Build per-core sorted edge lists, a shared iteration plan, and filled
    arrays. Returns (plan_iters [ntiles][...], idx_blocks [NCORES] list of
    int32 arrays, off_blocks [NCORES] list of f32 arrays)."""
    s_new = new_src[src_old].astype(np.int64)
    d_new = new_dst[dst_old].astype(np.int64)
    core = d_new // shard_dst
    slot = d_new % shard_dst
    per_core = []
    counts = np.zeros((NCORES, ntiles), dtype=np.int64)
    for c in range(NCORES):
        m = core == c
        sl = slot[m]
        o = np.argsort(sl, kind="stable")
        per_core.append((s_new[m][o].astype(np.int32), sl[o].astype(np.int32)))
        counts[c] = np.bincount(sl // P, minlength=ntiles)
    padded = ((counts.max(axis=0) + P - 1) // P * P).astype(np.int64)
    plans = []
    for k in range(ntiles):
        n128 = int(padded[k]) // P
        iters = []
        while n128 > 0:
            take = min(EW, n128)
            iters.append(take)
            n128 -= take
        plans.append(iters)
    idx_blocks = [[] for _ in range(NCORES)]
    off_blocks = [[] for _ in range(NCORES)]
    for c in range(NCORES):
        srcs, slots_c = per_core[c]
        pos = 0
        for k in range(ntiles):
            cnt = int(counts[c, k])
            n_pad = int(padded[k])
            s_arr = np.zeros(n_pad, np.int32)
            d_arr = np.zeros(n_pad, np.int32)
            o_arr = np.full(n_pad, -1.0, np.float32)
            s_arr[:cnt] = srcs[pos:pos + cnt]
            d_arr[:cnt] = slots_c[pos:pos + cnt]
            o_arr[:cnt] = (slots_c[pos:pos + cnt] - k * P).astype(np.float32)
            pos += cnt
            eoff = 0
            for S in plans[k]:
                blk = slice(eoff, eoff + S * P)
                sb = s_arr[blk].reshape(S, P).T            # [128, S]
                db = d_arr[blk].reshape(S, P).T
                ob = o_arr[blk].reshape(S, P).T
                idx_blk = np.stack([sb, db], axis=2).reshape(P, S * 2)
                idx_blocks[c].append(np.ascontiguousarray(idx_blk).ravel())
                off_blocks[c].append(np.ascontiguousarray(ob).ravel())
                eoff += S * P
    return plans, idx_blocks, off_blocks


# ================= device program =================

def _inproj(nc, sb, pe, wlin, blin, x0_ap, xh_ap, in_dim, ncols):
    """Feature-major input projection: xh[fo, :] = W.T @ x0 + b."""
    col_tiles = []
    j = 0
    while j < ncols:
        w = min(512, ncols - j)
        col_tiles.append((j, w))
        j += w
    for (j, w) in col_tiles:
        rhs = sb.tile([in_dim, 512], FP32, tag="ip_rhs")
        nc.sync.dma_start(out=rhs[:, :w], in_=x0_ap[:, j:j + w])
        for fo in range(2):
            ps = pe.tile([P, 512], FP32, tag="pe")
            nc.tensor.matmul(ps[:, :w], lhsT=wlin[:, fo * P:(fo + 1) * P],
                             rhs=rhs[:, :w], start=True, stop=True)
            ot = sb.tile([P, 512], FP32, tag="ip_out")
            nc.scalar.activation(ot[:, :w], ps[:, :w],
                                 mybir.ActivationFunctionType.Identity,
                                 bias=blin[:, fo:fo + 1])
            nc.sync.dma_start(out=xh_ap[fo * P:(fo + 1) * P, j:j + w], in_=ot[:, :w])


def _kv_phase(nc, sb, pe, wkv, src_chunks, kv_ap, ntiles):
    """kv[node, :] = x.T @ Wkv (no bias). src_chunks(nt, f) -> AP [128,128]."""
    for nt in range(ntiles):
        ps = pe.tile([P, KV_W], FP32, tag="pe")
        for f in range(2):
            lx = sb.tile([P, P], FP32, tag="kv_lx")
            nc.sync.dma_start(out=lx[:], in_=src_chunks(nt, f))
            nc.tensor.matmul(ps[:], lhsT=lx[:], rhs=wkv[:, f * KV_W:(f + 1) * KV_W],
                             start=(f == 0), stop=(f == 1))
        ot = sb.tile([P, KV_W], FP32, tag="kv_out")
        nc.scalar.copy(ot[:], ps[:])
        nc.sync.dma_start(out=kv_ap[nt * P:(nt + 1) * P, :], in_=ot[:])


def _q_phase(nc, sb, pe, wq, bq_row, ones1, x_sh_ap, q_ap, base, ntiles):
    for nt in range(ntiles):
        ps = pe.tile([P, HID], FP32, tag="pe")
        for f in range(2):
            lx = sb.tile([P, P], FP32, tag="q_lx")
            nc.sync.dma_start(out=lx[:], in_=x_sh_ap[f * P:(f + 1) * P,
                                                     base + nt * P: base + (nt + 1) * P])
            nc.tensor.matmul(ps[:], lhsT=lx[:], rhs=wq[:, f * HID:(f + 1) * HID],
                             start=(f == 0), stop=False)
        nc.tensor.matmul(ps[:], lhsT=ones1[:, :P], rhs=bq_row[:, :],
                         start=False, stop=True)
        ot = sb.tile([P, HID], FP32, tag="q_out")
        nc.scalar.copy(ot[:], ps[:])
        nc.sync.dma_start(out=q_ap[nt * P:(nt + 1) * P, :], in_=ot[:])


def _edge_phase(nc, sb, ed, pe_agg, tp, op, plans, arr_cursor,
                edge_idx_ap, edge_off_ap, kv_ap, q_ap, iota_f, identity,
                wout, bout, bv, skipc, x_sh_ap, xout_ap, col_base,
                ntiles):
    """Edge phase + epilogue + out-proj for one (layer, edge type)."""
    for k in range(ntiles):
        iters = plans[k]
        nsub_total = sum(iters)
        agg = pe_agg.tile([P, 260], FP32, tag="agg")
        sub_i = 0
        for S in iters:
            io, oo = arr_cursor[0], arr_cursor[1]
            arr_cursor[0] += P * S * 2
            arr_cursor[1] += P * S
            idx = ed.tile([P, EW * 2], I32, tag="idx")
            nc.sync.dma_start(out=idx[:, :S * 2],
                              in_=edge_idx_ap[io:io + P * S * 2].rearrange("(p x) -> p x", p=P))
            dstf = ed.tile([P, EW], FP32, tag="dstf")
            nc.sync.dma_start(out=dstf[:, :S],
                              in_=edge_off_ap[oo:oo + P * S].rearrange("(p x) -> p x", p=P))
            kvg = ed.tile([P, EW * KV_W], FP32, tag="kvg")
            qg = ed.tile([P, EW * HID], FP32, tag="qg")
            for s in range(S):
                nc.gpsimd.indirect_dma_start(
                    out=kvg[:, s * KV_W:(s + 1) * KV_W], out_offset=None,
                    in_=kv_ap,
                    in_offset=bass.IndirectOffsetOnAxis(ap=idx[:, 2 * s:2 * s + 1], axis=0))
                nc.gpsimd.indirect_dma_start(
                    out=qg[:, s * HID:(s + 1) * HID], out_offset=None,
                    in_=q_ap,
                    in_offset=bass.IndirectOffsetOnAxis(ap=idx[:, 2 * s + 1:2 * s + 2], axis=0))
            prod = ed.tile([P, EW * HID], FP32, tag="prod")
            kvg_k = kvg[:, :S * KV_W].rearrange("p (s x) -> p s x", x=KV_W)[:, :, 0:HID]
            nc.vector.tensor_tensor(out=prod[:, :S * HID], in0=kvg_k,
                                    in1=qg[:, :S * HID], op=mybir.AluOpType.mult)
            lg = ed.tile([P, EW * H], FP32, tag="lg")
            nc.vector.tensor_reduce(
                out=lg[:, :S * H],
                in_=prod[:, :S * HID].rearrange("p (x d) -> p x d", d=D),
                axis=mybir.AxisListType.X, op=mybir.AluOpType.add)
            ex = ed.tile([P, EW * H], FP32, tag="ex")
            nc.scalar.activation(ex[:, :S * H], lg[:, :S * H],
                                 mybir.ActivationFunctionType.Exp)
            rhs = ed.tile([P, EW * 260], FP32, tag="rhs")
            for s in range(S):
                for h in range(H):
                    nc.vector.tensor_scalar(
                        out=rhs[:, s * 260 + h * D:s * 260 + (h + 1) * D],
                        in0=kvg[:, s * KV_W + HID + h * D:s * KV_W + HID + (h + 1) * D],
                        scalar1=ex[:, s * H + h:s * H + h + 1], scalar2=None,
                        op0=mybir.AluOpType.mult)
            nc.vector.tensor_copy(
                out=rhs[:, :S * 260].rearrange("p (s x) -> p s x", x=260)[:, :, 256:260],
                in_=ex[:, :S * H].rearrange("p (s h) -> p s h", h=H))
            oneh = ed.tile([P, EW * P], FP32, tag="oneh")
            for s in range(S):
                nc.vector.tensor_tensor(
                    out=oneh[:, s * P:(s + 1) * P],
                    in0=dstf[:, s:s + 1].to_broadcast([P, P]),
                    in1=iota_f[:, :], op=mybir.AluOpType.is_equal)
            for s in range(S):
                nc.tensor.matmul(agg[:], lhsT=oneh[:, s * P:(s + 1) * P],
                                 rhs=rhs[:, s * 260:(s + 1) * 260],
                                 start=(sub_i == 0), stop=(sub_i == nsub_total - 1))
                sub_i += 1
        # ---- epilogue: normalize, gelu, out-proj, skip ----
        aggsb = sb.tile([P, HID], FP32, tag="aggsb")
        if nsub_total == 0:
            nc.vector.memset(aggsb[:], 0.0)
        else:
            sden = sb.tile([P, H], FP32, tag="sden")
            nc.vector.tensor_scalar(out=sden[:], in0=agg[:, 256:260],
                                    scalar1=1e-16, scalar2=None,
                                    op0=mybir.AluOpType.add)
            recip = sb.tile([P, H], FP32, tag="recip")
            nc.vector.reciprocal(recip[:], sden[:])
            for h in range(H):
                nc.scalar.activation(aggsb[:, h * D:(h + 1) * D],
                                     agg[:, h * D:(h + 1) * D],
                                     mybir.ActivationFunctionType.Copy,
                                     scale=recip[:, h:h + 1])
        gelu_fm = sb.tile([P, HID], FP32, tag="gelu_fm")
        for c in range(2):
            tpp = tp.tile([P, P], FP32, tag="tp")
            nc.tensor.transpose(tpp[:], aggsb[:, c * P:(c + 1) * P], identity[:])
            nc.scalar.activation(gelu_fm[:, c * P:(c + 1) * P], tpp[:],
                                 mybir.ActivationFunctionType.Gelu,
                                 bias=bv[:, c:c + 1])
        for fo in range(2):
            pso = op.tile([P, P], FP32, tag="op")
            for c in range(2):
                nc.tensor.matmul(pso[:], lhsT=wout[:, c * HID + fo * P:c * HID + (fo + 1) * P],
                                 rhs=gelu_fm[:, c * P:(c + 1) * P],
                                 start=(c == 0), stop=(c == 1))
            oo_t = sb.tile([P, P], FP32, tag="oo")
            nc.scalar.activation(oo_t[:], pso[:],
                                 mybir.ActivationFunctionType.Identity,
                                 bias=bout[:, fo:fo + 1])
            xs = sb.tile([P, P], FP32, tag="xs")
            nc.sync.dma_start(out=xs[:], in_=x_sh_ap[fo * P:(fo + 1) * P,
                                                     col_base + k * P: col_base + (k + 1) * P])
            xs2 = sb.tile([P, P], FP32, tag="xs2")
            nc.vector.tensor_scalar(out=xs2[:], in0=xs[:], scalar1=float(skipc),
                                    scalar2=None, op0=mybir.AluOpType.mult)
            nc.vector.tensor_tensor(out=oo_t[:], in0=oo_t[:], in1=xs2[:],
                                    op=mybir.AluOpType.add)
            nc.sync.dma_start(out=xout_ap[fo * P:(fo + 1) * P,
                                          col_base + k * P: col_base + (k + 1) * P],
                              in_=oo_t[:])


def _build_program(plans_t, plans_r, n_edge_idx, n_edge_off, trace_scopes=False):
    nc = bacc.Bacc("TRN2", target_bir_lowering=False, debug=False,
                   enable_asserts=False, num_devices=NCORES)
    dt = FP32
    # ---- I/O ----
    x0p = nc.dram_tensor("x0p", [IN_P, NP_PAD], dt, kind="ExternalInput").ap()
    x0d = nc.dram_tensor("x0d", [IN_D, ND_PAD], dt, kind="ExternalInput").ap()
    x0p_sh = nc.dram_tensor("x0p_sh", [IN_P, SH_P], dt, kind="ExternalInput").ap()
    x0d_sh = nc.dram_tensor("x0d_sh", [IN_D, SH_D], dt, kind="ExternalInput").ap()
    edge_idx = nc.dram_tensor("edge_idx", [n_edge_idx], I32, kind="ExternalInput").ap()
    edge_off = nc.dram_tensor("edge_off", [n_edge_off], dt, kind="ExternalInput").ap()
    win = {}
    for t, ind in (("p", IN_P), ("d", IN_D)):
        win[f"wlin_{t}"] = nc.dram_tensor(f"wlin_{t}", [ind, HID], dt, kind="ExternalInput").ap()
        win[f"blin_{t}"] = nc.dram_tensor(f"blin_{t}", [P, 2], dt, kind="ExternalInput").ap()
    for L in range(2):
        for t in ("p", "d"):
            win[f"wkv_{t}_{L}"] = nc.dram_tensor(f"wkv_{t}_{L}", [P, 2 * KV_W], dt, kind="ExternalInput").ap()
            win[f"wq_{t}_{L}"] = nc.dram_tensor(f"wq_{t}_{L}", [P, 2 * HID], dt, kind="ExternalInput").ap()
            win[f"bq_{t}_{L}"] = nc.dram_tensor(f"bq_{t}_{L}", [1, HID], dt, kind="ExternalInput").ap()
            win[f"wout_{t}_{L}"] = nc.dram_tensor(f"wout_{t}_{L}", [P, 2 * HID], dt, kind="ExternalInput").ap()
            win[f"bout_{t}_{L}"] = nc.dram_tensor(f"bout_{t}_{L}", [P, 2], dt, kind="ExternalInput").ap()
            win[f"bv_{t}_{L}"] = nc.dram_tensor(f"bv_{t}_{L}", [P, 2], dt, kind="ExternalInput").ap()
    out_sh = nc.dram_tensor("out_sh", [HID, SH_ALL], dt, kind="ExternalOutput").ap()
    # ---- DRAM scratch ----
    xh_p = nc.dram_tensor("xh_p", [HID, NP_PAD], dt, kind="Internal").ap()
    xh_d = nc.dram_tensor("xh_d", [HID, ND_PAD], dt, kind="Internal").ap()
    xh_sh = nc.dram_tensor("xh_sh", [HID, SH_ALL], dt, kind="Internal").ap()
    kv_p = nc.dram_tensor("kv_p", [NP_PAD, KV_W], dt, kind="Internal").ap()
    kv_d = nc.dram_tensor("kv_d", [ND_PAD, KV_W], dt, kind="Internal").ap()
    qt_p = nc.dram_tensor("qt_p", [SH_P, HID], dt, kind="Internal").ap()
    qt_d = nc.dram_tensor("qt_d", [SH_D, HID], dt, kind="Internal").ap()
    x1_sh = nc.dram_tensor("x1_sh", [HID, SH_ALL], dt, kind="Internal").ap()
    x1_full = nc.dram_tensor("x1_full", [NCORES, HID, SH_ALL], dt, kind="Internal",
                             addr_space="Shared").ap()

    with tile.TileContext(nc) as tc:
        with (
            tc.tile_pool(name="const", bufs=1) as cp,
            tc.tile_pool(name="sb", bufs=3) as sb,
            tc.tile_pool(name="ed", bufs=3) as ed,
            tc.tile_pool(name="pe", bufs=2, space="PSUM") as pe,
            tc.tile_pool(name="agg", bufs=2, space="PSUM") as pe_agg,
            tc.tile_pool(name="tp", bufs=2, space="PSUM") as tp,
            tc.tile_pool(name="op", bufs=2, space="PSUM") as op,
        ):
            # consts
            identity = cp.tile([P, P], FP32)
            make_identity(nc, identity[:])
            iota_i = cp.tile([P, P], I32)
            nc.gpsimd.iota(iota_i[:], pattern=[[1, P]], base=0, channel_multiplier=0)
            iota_f = cp.tile([P, P], FP32)
            nc.vector.tensor_copy(iota_f[:], iota_i[:])
            ones1 = cp.tile([1, P], FP32)
            nc.vector.memset(ones1[:], 1.0)
            # weights to SBUF
            wsb = {}
            for name, ap in win.items():
                t_ = cp.tile(list(ap.shape), FP32, tag=name)
                nc.sync.dma_start(out=t_[:], in_=ap)
                wsb[name] = t_

            # ---- Phase A: input projections ----
            _inproj(nc, sb, pe, wsb["wlin_p"], wsb["blin_p"], x0p, xh_p, IN_P, NP_PAD)
            _inproj(nc, sb, pe, wsb["wlin_d"], wsb["blin_d"], x0d, xh_d, IN_D, ND_PAD)
            _inproj(nc, sb, pe, wsb["wlin_p"], wsb["blin_p"], x0p_sh,
                    xh_sh[:, 0:SH_P], IN_P, SH_P)
            _inproj(nc, sb, pe, wsb["wlin_d"], wsb["blin_d"], x0d_sh,
                    xh_sh[:, SH_P:SH_ALL], IN_D, SH_D)

            for L in range(2):
                x_sh_ap = xh_sh if L == 0 else x1_sh
                xout_ap = x1_sh if L == 0 else out_sh
                if L == 0:
                    src_p = lambda nt, f: xh_p[f * P:(f + 1) * P, nt * P:(nt + 1) * P]
                    src_d = lambda nt, f: xh_d[f * P:(f + 1) * P, nt * P:(nt + 1) * P]
                else:
                    def src_p(nt, f):
                        b, k = nt // NT_P, nt % NT_P
                        return x1_full[b, f * P:(f + 1) * P, k * P:(k + 1) * P]
                    def src_d(nt, f):
                        b, k = nt // NT_D, nt % NT_D
                        return x1_full[b, f * P:(f + 1) * P, SH_P + k * P:SH_P + (k + 1) * P]
                _kv_phase(nc, sb, pe, wsb[f"wkv_p_{L}"], src_p, kv_p, NP_PAD // P)
                _kv_phase(nc, sb, pe, wsb[f"wkv_d_{L}"], src_d, kv_d, ND_PAD // P)
                _q_phase(nc, sb, pe, wsb[f"wq_p_{L}"], wsb[f"bq_p_{L}"], ones1,
                         x_sh_ap, qt_p, 0, NT_P)
                _q_phase(nc, sb, pe, wsb[f"wq_d_{L}"], wsb[f"bq_d_{L}"], ones1,
                         x_sh_ap, qt_d, SH_P, NT_D)
                arr_cursor = [0, 0]
                # takes: src=patient kv, dst=drug
                _edge_phase(nc, sb, ed, pe_agg, tp, op, plans_t, arr_cursor,
                            edge_idx, edge_off, kv_p, qt_d, iota_f, identity,
                            wsb[f"wout_d_{L}"], wsb[f"bout_d_{L}"], wsb[f"bv_d_{L}"],
                            SKIPC[f"d_{L}"], x_sh_ap, xout_ap, SH_P, NT_D)
                # rev: src=drug kv, dst=patient
                _edge_phase(nc, sb, ed, pe_agg, tp, op, plans_r, arr_cursor,
                            edge_idx, edge_off, kv_d, qt_p, iota_f, identity,
                            wsb[f"wout_p_{L}"], wsb[f"bout_p_{L}"], wsb[f"bv_p_{L}"],
                            SKIPC[f"p_{L}"], x_sh_ap, xout_ap, 0, NT_P)
                if L == 0:
                    nc.gpsimd.collective_compute(
                        "AllGather", mybir.AluOpType.bypass,
                        replica_groups=[list(range(NCORES))],
                        ins=[x1_sh.opt()], outs=[x1_full.opt()])
    return nc


SKIPC = {}  # filled by kernel() before _build_program
LAST_EXEC_NS = None


def prepare(x_patient, x_drug, ei_takes_src, ei_takes_dst, ei_rev_src, ei_rev_dst, params):
    """Build (nc, in_maps, assemble) without running."""
    global SKIPC
    x_patient = np.asarray(x_patient, np.float32)
    x_drug = np.asarray(x_drug, np.float32)
    W = _fuse_weights(params)
    SKIPC = {f"{t}_{L}": W[f"skipc_{t}_{L}"] for t in ("p", "d") for L in range(2)}

    deg_d = np.bincount(np.asarray(ei_takes_dst), minlength=N_D)
    deg_p = np.bincount(np.asarray(ei_rev_dst), minlength=N_P)
    new_p = _shard_perm(deg_p, N_P, SH_P)
    new_d = _shard_perm(deg_d, N_D, SH_D)

    plans_t, idx_t, off_t = _plan_and_fill(
        np.asarray(ei_takes_src), np.asarray(ei_takes_dst), new_p, new_d, SH_D, NT_D)
    plans_r, idx_r, off_r = _plan_and_fill(
        np.asarray(ei_rev_src), np.asarray(ei_rev_dst), new_d, new_p, SH_P, NT_P)

    edge_idx_c, edge_off_c = [], []
    for c in range(NCORES):
        edge_idx_c.append(np.concatenate(idx_t[c] + idx_r[c]).astype(np.int32))
        edge_off_c.append(np.concatenate(off_t[c] + off_r[c]).astype(np.float32))
    n_edge_idx = len(edge_idx_c[0])
    n_edge_off = len(edge_off_c[0])

    # permuted feature-major inputs
    xp = np.zeros((NP_PAD, IN_P), np.float32)
    xp[new_p] = x_patient
    x0p = np.ascontiguousarray(xp.T)
    xd = np.zeros((ND_PAD, IN_D), np.float32)
    xd[new_d] = x_drug
    x0d = np.ascontiguousarray(xd.T)

    nc = _build_program(plans_t, plans_r, n_edge_idx, n_edge_off)

    in_maps = []
    for c in range(NCORES):
        m = {
            "x0p": x0p, "x0d": x0d,
            "x0p_sh": np.ascontiguousarray(x0p[:, c * SH_P:(c + 1) * SH_P]),
            "x0d_sh": np.ascontiguousarray(x0d[:, c * SH_D:(c + 1) * SH_D]),
            "edge_idx": edge_idx_c[c], "edge_off": edge_off_c[c],
        }
        for name in ("wlin_p", "wlin_d", "blin_p", "blin_d"):
            m[name] = W[name]
        for L in range(2):
            for t in ("p", "d"):
                for stem in ("wkv", "wq", "bq", "wout", "bout", "bv"):
                    name = f"{stem}_{t}_{L}"
                    m[name] = W[name]
        in_maps.append(m)

    nc.compile()

    def assemble(outs):
        full_p = np.concatenate([o[:, :SH_P] for o in outs], axis=1)   # [256, NP_PAD]
        full_d = np.concatenate([o[:, SH_P:] for o in outs], axis=1)   # [256, ND_PAD]
        out_p = full_p.T[new_p].astype(np.float32)
        out_d = full_d.T[new_d].astype(np.float32)
        return (out_p, out_d)

    return nc, in_maps, assemble


def kernel(**inputs):
    nc, in_maps, assemble = prepare(**inputs)
    res = bass_utils.run_bass_kernel_spmd(nc, in_maps, core_ids=list(range(NCORES)))
    return assemble([r["out_sh"] for r in res.results])


if __name__ == "__main__":
    # tiny smoke: random inputs with the real shapes
    rng = np.random.default_rng(0)
    print("smoke run only checks compile+exec plumbing")


# revision 7
# speedup vs baseline: 1.0465x; 1.0465x over previous
"""2-layer HGT (patient/drug) kernel for 8x TRN2 NeuronCores via Bass/Tile.

Strategy:
  - Host: fuse per-head relation matrices R_k/R_v into the K/V projection
    weights, fold p_rel*scale into Q weights, fold sigmoid(skip) into the
    output projection. Permute nodes so destination nodes are degree-sorted
    and dealt round-robin across cores (balanced edges AND nodes); each core
    owns a contiguous shard of permuted node ids.
  - Device (SPMD, one program): feature-major dense projections (TensorE),
    per-edge gather of fused KV rows + Q rows (indirect DMA), logits by
    elementwise mul + segmented reduce (DVE), exp (ACT, no max-subtraction
    needed: softmax is shift-invariant and logits are O(1) here), and
    segment-sum via one-hot selection matmuls accumulating in PSUM.
    One AllGather of node features between the two layers.
"""

import os
import numpy as np

import concourse.bacc as bacc
import concourse.bass as bass
import concourse.mybir as mybir
import concourse.tile as tile
from concourse import bass_utils
from concourse.masks import make_identity

FP32 = mybir.dt.float32
I32 = mybir.dt.int32

# ---- problem constants (hardcoded per contract) ----
N_P, N_D = 50000, 5000
IN_P, IN_D = 128, 64
HID, H, D = 256, 4, 64
NCORES = 8
P = 128
SH_P = 6272          # patient shard cols per core (49 tiles of 128)
SH_D = 640           # drug shard cols per core (5 tiles)
NP_PAD = SH_P * NCORES   # 50176
ND_PAD = SH_D * NCORES   # 5120
NT_P = SH_P // P     # 49
NT_D = SH_D // P     # 5
EW = 4               # max 128-edge subgroups per iteration
SH_ALL = SH_P + SH_D  # 6912

KV_W = 2 * HID       # 512 fused (k_rel | v_rel) row width


# ================= host-side preprocessing =================

def _fuse_weights(params):
    """Returns dict of numpy arrays for device consumption."""
    out = {}
    scale = np.float32(1.0 / np.sqrt(D))
    # input projections
    for t, key in (("p", "patient"), ("d", "drug")):
        W = np.asarray(params["lin"][key]["W"], np.float32)
        b = np.asarray(params["lin"][key]["b"], np.float32)
        out[f"wlin_{t}"] = np.ascontiguousarray(W)
        out[f"blin_{t}"] = np.ascontiguousarray(b.reshape(2, 128).T)  # [128,2] col=fo chunk
    et_src = {"p": "patient__takes__drug", "d": "drug__rev_takes__patient"}
    et_dst = {"p": "drug__rev_takes__patient", "d": "patient__takes__drug"}
    for L, lp in enumerate(params["layers"]):
        for t, key in (("p", "patient"), ("d", "drug")):
            W = np.asarray(lp["kqv"][key]["W"], np.float32)   # [256, 768]
            b = np.asarray(lp["kqv"][key]["b"], np.float32)   # [768]
            Wk, Wq, Wv = W[:, :HID], W[:, HID:2 * HID], W[:, 2 * HID:]
            bk, bq, bv = b[:HID], b[HID:2 * HID], b[2 * HID:]
            Rk = np.asarray(lp["k_rel"][et_src[t]], np.float32)  # [H, D, D]
            Rv = np.asarray(lp["v_rel"][et_src[t]], np.float32)
            prel = np.asarray(lp["p_rel"][et_dst[t]], np.float32)  # [H]
            # k_rel fused (bias dropped: per-dst softmax shift invariance)
            Wk4 = Wk.reshape(HID, H, D)
            Wkf = np.einsum("fhd,hde->fhe", Wk4, Rk).reshape(HID, HID)
            Wv4 = Wv.reshape(HID, H, D)
            Wvf = np.einsum("fhd,hde->fhe", Wv4, Rv).reshape(HID, HID)
            bvf = np.einsum("hd,hde->he", bv.reshape(H, D), Rv).reshape(HID)
            # q scaled by p_rel*scale per head
            qs = (prel * scale).repeat(D)  # [256]
            Wqf = Wq * qs[None, :]
            bqf = bq * qs
            sk = 1.0 / (1.0 + np.exp(-np.float64(lp["skip"][key])))
            sk = np.float32(sk)
            Wo = np.asarray(lp["out"][key]["W"], np.float32) * sk
            bo = np.asarray(lp["out"][key]["b"], np.float32) * sk
            wkv = np.concatenate([Wkf, Wvf], axis=1)  # [256,512]
            out[f"wkv_{t}_{L}"] = np.ascontiguousarray(
                wkv.reshape(2, 128, KV_W).transpose(1, 0, 2).reshape(128, 2 * KV_W))
            out[f"wq_{t}_{L}"] = np.ascontiguousarray(
                Wqf.reshape(2, 128, HID).transpose(1, 0, 2).reshape(128, 2 * HID))
            out[f"bq_{t}_{L}"] = np.ascontiguousarray(bqf.reshape(1, HID))  # [1,256]
            out[f"wout_{t}_{L}"] = np.ascontiguousarray(
                Wo.reshape(2, 128, HID).transpose(1, 0, 2).reshape(128, 2 * HID))
            out[f"bout_{t}_{L}"] = np.ascontiguousarray(bo.reshape(2, 128).T)  # [128,2]
            out[f"bv_{t}_{L}"] = np.ascontiguousarray(bvf.reshape(2, 128).T)   # [128,2]
            out[f"skipc_{t}_{L}"] = np.float32(1.0 - sk)  # python-side const
    return out


def _shard_perm(deg, n, shard):
    """Degree-sorted round-robin dealing. Returns new_of_old [n] -> new id."""
    order = np.argsort(-deg, kind="stable")
    idx = np.arange(n, dtype=np.int64)
    new_ids = (idx % NCORES) * shard + idx // NCORES
    new_of_old = np.empty(n, dtype=np.int64)
    new_of_old[order] = new_ids
    return new_of_old


def _plan_and_fill(src_old, dst_old, new_src, new_dst, shard_dst, ntiles):
    """Build per-core sorted edge lists, a shared iteration plan, and filled
    arrays. Returns (plan_iters [ntiles][...], idx_blocks [NCORES] list of
    int32 arrays, off_blocks [NCORES] list of f32 arrays)."""
    s_new = new_src[src_old].astype(np.int64)
    d_new = new_dst[dst_old].astype(np.int64)
    core = d_new // shard_dst
    slot = d_new % shard_dst
    per_core = []
    counts = np.zeros((NCORES, ntiles), dtype=np.int64)
    for c in range(NCORES):
        m = core == c
        sl = slot[m]
        o = np.argsort(sl, kind="stable")
        per_core.append((s_new[m][o].astype(np.int32), sl[o].astype(np.int32)))
        counts[c] = np.bincount(sl // P, minlength=ntiles)
    padded = ((counts.max(axis=0) + P - 1) // P * P).astype(np.int64)
    plans = []
    for k in range(ntiles):
        n128 = int(padded[k]) // P
        iters = []
        while n128 > 0:
            take = min(EW, n128)
            iters.append(take)
            n128 -= take
        plans.append(iters)
    idx_blocks = [[] for _ in range(NCORES)]
    off_blocks = [[] for _ in range(NCORES)]
    for c in range(NCORES):
        srcs, slots_c = per_core[c]
        pos = 0
        for k in range(ntiles):
            cnt = int(counts[c, k])
            n_pad = int(padded[k])
            s_arr = np.zeros(n_pad, np.int32)
            d_arr = np.zeros(n_pad, np.int32)
            o_arr = np.full(n_pad, -1.0, np.float32)
            s_arr[:cnt] = srcs[pos:pos + cnt]
            d_arr[:cnt] = slots_c[pos:pos + cnt]
            o_arr[:cnt] = (slots_c[pos:pos + cnt] - k * P).astype(np.float32)
            pos += cnt
            eoff = 0
            for S in plans[k]:
                blk = slice(eoff, eoff + S * P)
                sb = s_arr[blk].reshape(S, P).T            # [128, S]
                db = d_arr[blk].reshape(S, P).T
                ob = o_arr[blk].reshape(S, P).T
                idx_blk = np.stack([sb, db], axis=2).reshape(P, S * 2)
                idx_blocks[c].append(np.ascontiguousarray(idx_blk).ravel())
                off_blocks[c].append(np.ascontiguousarray(ob).ravel())
                eoff += S * P
    return plans, idx_blocks, off_blocks


# ================= device program =================

def _inproj(nc, sb, pe, wlin, blin, x0_ap, xh_ap, in_dim, ncols):
    """Feature-major input projection: xh[fo, :] = W.T @ x0 + b."""
    col_tiles = []
    j = 0
    while j < ncols:
        w = min(512, ncols - j)
        col_tiles.append((j, w))
        j += w
    for (j, w) in col_tiles:
        rhs = sb.tile([in_dim, 512], FP32, tag="ip_rhs")
        nc.sync.dma_start(out=rhs[:, :w], in_=x0_ap[:, j:j + w])
        for fo in range(2):
            ps = pe.tile([P, 512], FP32, tag="pe")
            nc.tensor.matmul(ps[:, :w], lhsT=wlin[:, fo * P:(fo + 1) * P],
                             rhs=rhs[:, :w], start=True, stop=True)
            ot = sb.tile([P, 512], FP32, tag="ip_out")
            nc.scalar.activation(ot[:, :w], ps[:, :w],
                                 mybir.ActivationFunctionType.Identity,
                                 bias=blin[:, fo:fo + 1])
            nc.sync.dma_start(out=xh_ap[fo * P:(fo + 1) * P, j:j + w], in_=ot[:, :w])


def _kv_phase(nc, sb, pe, wkv, src_chunks, kv_ap, ntiles):
    """kv[node, :] = x.T @ Wkv (no bias). src_chunks(nt, f) -> AP [128,128]."""
    for nt in range(ntiles):
        ps = pe.tile([P, KV_W], FP32, tag="pe")
        for f in range(2):
            lx = sb.tile([P, P], FP32, tag="kv_lx")
            nc.sync.dma_start(out=lx[:], in_=src_chunks(nt, f))
            nc.tensor.matmul(ps[:], lhsT=lx[:], rhs=wkv[:, f * KV_W:(f + 1) * KV_W],
                             start=(f == 0), stop=(f == 1))
        ot = sb.tile([P, KV_W], FP32, tag="kv_out")
        nc.scalar.copy(ot[:], ps[:])
        nc.sync.dma_start(out=kv_ap[nt * P:(nt + 1) * P, :], in_=ot[:])


def _q_phase(nc, sb, pe, wq, bq_row, ones1, x_sh_ap, q_ap, base, ntiles):
    for nt in range(ntiles):
        ps = pe.tile([P, HID], FP32, tag="pe")
        for f in range(2):
            lx = sb.tile([P, P], FP32, tag="q_lx")
            nc.sync.dma_start(out=lx[:], in_=x_sh_ap[f * P:(f + 1) * P,
                                                     base + nt * P: base + (nt + 1) * P])
            nc.tensor.matmul(ps[:], lhsT=lx[:], rhs=wq[:, f * HID:(f + 1) * HID],
                             start=(f == 0), stop=False)
        nc.tensor.matmul(ps[:], lhsT=ones1[:, :P], rhs=bq_row[:, :],
                         start=False, stop=True)
        ot = sb.tile([P, HID], FP32, tag="q_out")
        nc.scalar.copy(ot[:], ps[:])
        nc.sync.dma_start(out=q_ap[nt * P:(nt + 1) * P, :], in_=ot[:])


def _edge_phase(nc, sb, ed, pe_agg, tp, op, plans, arr_cursor,
                edge_idx_ap, edge_off_ap, kv_ap, q_ap, iota_f, identity,
                wout, bout, bv, skipc, x_sh_ap, xout_ap, col_base,
                ntiles):
    """Edge phase + epilogue + out-proj for one (layer, edge type)."""
    for k in range(ntiles):
        iters = plans[k]
        if "e" in os.environ.get("KERNEL_SKIP", ""):
            iters = []
        nsub_total = sum(iters)
        agg = pe_agg.tile([P, 260], FP32, tag="agg")
        sub_i = 0
        for S in iters:
            io, oo = arr_cursor[0], arr_cursor[1]
            arr_cursor[0] += P * S * 2
            arr_cursor[1] += P * S
            idx = ed.tile([P, EW * 2], I32, tag="idx")
            nc.sync.dma_start(out=idx[:, :S * 2],
                              in_=edge_idx_ap[io:io + P * S * 2].rearrange("(p x) -> p x", p=P))
            dstf = ed.tile([P, EW], FP32, tag="dstf")
            nc.sync.dma_start(out=dstf[:, :S],
                              in_=edge_off_ap[oo:oo + P * S].rearrange("(p x) -> p x", p=P))
            kvg = ed.tile([P, EW * KV_W], FP32, tag="kvg")
            qg = ed.tile([P, EW * HID], FP32, tag="qg")
            for s in range(S):
                nc.gpsimd.indirect_dma_start(
                    out=kvg[:, s * KV_W:(s + 1) * KV_W], out_offset=None,
                    in_=kv_ap,
                    in_offset=bass.IndirectOffsetOnAxis(ap=idx[:, 2 * s:2 * s + 1], axis=0))
                if "g" in os.environ.get("KERNEL_SKIP", ""):
                    pass
                else:
                    nc.gpsimd.indirect_dma_start(
                        out=qg[:, s * HID:(s + 1) * HID], out_offset=None,
                        in_=q_ap,
                        in_offset=bass.IndirectOffsetOnAxis(ap=idx[:, 2 * s + 1:2 * s + 2], axis=0))
            prod = ed.tile([P, EW * HID], FP32, tag="prod")
            kvg_k = kvg[:, :S * KV_W].rearrange("p (s x) -> p s x", x=KV_W)[:, :, 0:HID]
            qg_in = qg[:, :S * HID] if "g" not in os.environ.get("KERNEL_SKIP", "") else \
                kvg[:, :S * KV_W].rearrange("p (s x) -> p s x", x=KV_W)[:, :, HID:2 * HID]
            nc.vector.tensor_tensor(out=prod[:, :S * HID], in0=kvg_k,
                                    in1=qg_in, op=mybir.AluOpType.mult)
            lg = ed.tile([P, EW * H], FP32, tag="lg")
            nc.vector.tensor_reduce(
                out=lg[:, :S * H],
                in_=prod[:, :S * HID].rearrange("p (x d) -> p x d", d=D),
                axis=mybir.AxisListType.X, op=mybir.AluOpType.add)
            ex = ed.tile([P, EW * H], FP32, tag="ex")
            nc.scalar.activation(ex[:, :S * H], lg[:, :S * H],
                                 mybir.ActivationFunctionType.Exp)
            rhs = ed.tile([P, EW * 260], FP32, tag="rhs")
            for s in range(S):
                for h in range(H):
                    nc.vector.tensor_scalar(
                        out=rhs[:, s * 260 + h * D:s * 260 + (h + 1) * D],
                        in0=kvg[:, s * KV_W + HID + h * D:s * KV_W + HID + (h + 1) * D],
                        scalar1=ex[:, s * H + h:s * H + h + 1], scalar2=None,
                        op0=mybir.AluOpType.mult)
            nc.vector.tensor_copy(
                out=rhs[:, :S * 260].rearrange("p (s x) -> p s x", x=260)[:, :, 256:260],
                in_=ex[:, :S * H].rearrange("p (s h) -> p s h", h=H))
            oneh = ed.tile([P, EW * P], FP32, tag="oneh")
            for s in range(S):
                nc.vector.tensor_tensor(
                    out=oneh[:, s * P:(s + 1) * P],
                    in0=dstf[:, s:s + 1].to_broadcast([P, P]),
                    in1=iota_f[:, :], op=mybir.AluOpType.is_equal)
            for s in range(S):
                nc.tensor.matmul(agg[:], lhsT=oneh[:, s * P:(s + 1) * P],
                                 rhs=rhs[:, s * 260:(s + 1) * 260],
                                 start=(sub_i == 0), stop=(sub_i == nsub_total - 1))
                sub_i += 1
        # ---- epilogue: normalize, gelu, out-proj, skip ----
        aggsb = sb.tile([P, HID], FP32, tag="aggsb")
        if nsub_total == 0:
            nc.vector.memset(aggsb[:], 0.0)
        else:
            sden = sb.tile([P, H], FP32, tag="sden")
            nc.vector.tensor_scalar(out=sden[:], in0=agg[:, 256:260],
                                    scalar1=1e-16, scalar2=None,
                                    op0=mybir.AluOpType.add)
            recip = sb.tile([P, H], FP32, tag="recip")
            nc.vector.reciprocal(recip[:], sden[:])
            for h in range(H):
                nc.scalar.activation(aggsb[:, h * D:(h + 1) * D],
                                     agg[:, h * D:(h + 1) * D],
                                     mybir.ActivationFunctionType.Copy,
                                     scale=recip[:, h:h + 1])
        gelu_fm = sb.tile([P, HID], FP32, tag="gelu_fm")
        for c in range(2):
            tpp = tp.tile([P, P], FP32, tag="tp")
            nc.tensor.transpose(tpp[:], aggsb[:, c * P:(c + 1) * P], identity[:])
            nc.scalar.activation(gelu_fm[:, c * P:(c + 1) * P], tpp[:],
                                 mybir.ActivationFunctionType.Gelu,
                                 bias=bv[:, c:c + 1])
        for fo in range(2):
            pso = op.tile([P, P], FP32, tag="op")
            for c in range(2):
                nc.tensor.matmul(pso[:], lhsT=wout[:, c * HID + fo * P:c * HID + (fo + 1) * P],
                                 rhs=gelu_fm[:, c * P:(c + 1) * P],
                                 start=(c == 0), stop=(c == 1))
            oo_t = sb.tile([P, P], FP32, tag="oo")
            nc.scalar.activation(oo_t[:], pso[:],
                                 mybir.ActivationFunctionType.Identity,
                                 bias=bout[:, fo:fo + 1])
            xs = sb.tile([P, P], FP32, tag="xs")
            nc.sync.dma_start(out=xs[:], in_=x_sh_ap[fo * P:(fo + 1) * P,
                                                     col_base + k * P: col_base + (k + 1) * P])
            xs2 = sb.tile([P, P], FP32, tag="xs2")
            nc.vector.tensor_scalar(out=xs2[:], in0=xs[:], scalar1=float(skipc),
                                    scalar2=None, op0=mybir.AluOpType.mult)
            nc.vector.tensor_tensor(out=oo_t[:], in0=oo_t[:], in1=xs2[:],
                                    op=mybir.AluOpType.add)
            nc.sync.dma_start(out=xout_ap[fo * P:(fo + 1) * P,
                                          col_base + k * P: col_base + (k + 1) * P],
                              in_=oo_t[:])


def _build_program(plans_t, plans_r, n_edge_idx, n_edge_off, trace_scopes=False):
    nc = bacc.Bacc("TRN2", target_bir_lowering=False, debug=False,
                   enable_asserts=False, num_devices=NCORES)
    dt = FP32
    # ---- I/O ----
    x0p = nc.dram_tensor("x0p", [IN_P, NP_PAD], dt, kind="ExternalInput").ap()
    x0d = nc.dram_tensor("x0d", [IN_D, ND_PAD], dt, kind="ExternalInput").ap()
    x0p_sh = nc.dram_tensor("x0p_sh", [IN_P, SH_P], dt, kind="ExternalInput").ap()
    x0d_sh = nc.dram_tensor("x0d_sh", [IN_D, SH_D], dt, kind="ExternalInput").ap()
    edge_idx = nc.dram_tensor("edge_idx", [n_edge_idx], I32, kind="ExternalInput").ap()
    edge_off = nc.dram_tensor("edge_off", [n_edge_off], dt, kind="ExternalInput").ap()
    win = {}
    for t, ind in (("p", IN_P), ("d", IN_D)):
        win[f"wlin_{t}"] = nc.dram_tensor(f"wlin_{t}", [ind, HID], dt, kind="ExternalInput").ap()
        win[f"blin_{t}"] = nc.dram_tensor(f"blin_{t}", [P, 2], dt, kind="ExternalInput").ap()
    for L in range(2):
        for t in ("p", "d"):
            win[f"wkv_{t}_{L}"] = nc.dram_tensor(f"wkv_{t}_{L}", [P, 2 * KV_W], dt, kind="ExternalInput").ap()
            win[f"wq_{t}_{L}"] = nc.dram_tensor(f"wq_{t}_{L}", [P, 2 * HID], dt, kind="ExternalInput").ap()
            win[f"bq_{t}_{L}"] = nc.dram_tensor(f"bq_{t}_{L}", [1, HID], dt, kind="ExternalInput").ap()
            win[f"wout_{t}_{L}"] = nc.dram_tensor(f"wout_{t}_{L}", [P, 2 * HID], dt, kind="ExternalInput").ap()
            win[f"bout_{t}_{L}"] = nc.dram_tensor(f"bout_{t}_{L}", [P, 2], dt, kind="ExternalInput").ap()
            win[f"bv_{t}_{L}"] = nc.dram_tensor(f"bv_{t}_{L}", [P, 2], dt, kind="ExternalInput").ap()
    out_sh = nc.dram_tensor("out_sh", [HID, SH_ALL], dt, kind="ExternalOutput").ap()
    # ---- DRAM scratch ----
    xh_p = nc.dram_tensor("xh_p", [HID, NP_PAD], dt, kind="Internal").ap()
    xh_d = nc.dram_tensor("xh_d", [HID, ND_PAD], dt, kind="Internal").ap()
    xh_sh = nc.dram_tensor("xh_sh", [HID, SH_ALL], dt, kind="Internal").ap()
    kv_p = nc.dram_tensor("kv_p", [NP_PAD, KV_W], dt, kind="Internal").ap()
    kv_d = nc.dram_tensor("kv_d", [ND_PAD, KV_W], dt, kind="Internal").ap()
    qt_p = nc.dram_tensor("qt_p", [SH_P, HID], dt, kind="Internal").ap()
    qt_d = nc.dram_tensor("qt_d", [SH_D, HID], dt, kind="Internal").ap()
    x1_sh = nc.dram_tensor("x1_sh", [HID, SH_ALL], dt, kind="Internal").ap()
    x1_full = nc.dram_tensor("x1_full", [NCORES, HID, SH_ALL], dt, kind="Internal",
                             addr_space="Shared").ap()

    with tile.TileContext(nc) as tc:
        with (
            tc.tile_pool(name="const", bufs=1) as cp,
            tc.tile_pool(name="sb", bufs=3) as sb,
            tc.tile_pool(name="ed", bufs=3) as ed,
            tc.tile_pool(name="pe", bufs=2, space="PSUM") as pe,
            tc.tile_pool(name="agg", bufs=2, space="PSUM") as pe_agg,
            tc.tile_pool(name="tp", bufs=2, space="PSUM") as tp,
            tc.tile_pool(name="op", bufs=2, space="PSUM") as op,
        ):
            # consts
            identity = cp.tile([P, P], FP32)
            make_identity(nc, identity[:])
            iota_i = cp.tile([P, P], I32)
            nc.gpsimd.iota(iota_i[:], pattern=[[1, P]], base=0, channel_multiplier=0)
            iota_f = cp.tile([P, P], FP32)
            nc.vector.tensor_copy(iota_f[:], iota_i[:])
            ones1 = cp.tile([1, P], FP32)
            nc.vector.memset(ones1[:], 1.0)
            # weights to SBUF
            wsb = {}
            for name, ap in win.items():
                t_ = cp.tile(list(ap.shape), FP32, tag=name)
                nc.sync.dma_start(out=t_[:], in_=ap)
                wsb[name] = t_

            # ---- Phase A: input projections ----
            _inproj(nc, sb, pe, wsb["wlin_p"], wsb["blin_p"], x0p, xh_p, IN_P, NP_PAD)
            _inproj(nc, sb, pe, wsb["wlin_d"], wsb["blin_d"], x0d, xh_d, IN_D, ND_PAD)
            _inproj(nc, sb, pe, wsb["wlin_p"], wsb["blin_p"], x0p_sh,
                    xh_sh[:, 0:SH_P], IN_P, SH_P)
            _inproj(nc, sb, pe, wsb["wlin_d"], wsb["blin_d"], x0d_sh,
                    xh_sh[:, SH_P:SH_ALL], IN_D, SH_D)

            for L in range(2):
                x_sh_ap = xh_sh if L == 0 else x1_sh
                xout_ap = x1_sh if L == 0 else out_sh
                if L == 0:
                    src_p = lambda nt, f: xh_p[f * P:(f + 1) * P, nt * P:(nt + 1) * P]
                    src_d = lambda nt, f: xh_d[f * P:(f + 1) * P, nt * P:(nt + 1) * P]
                else:
                    def src_p(nt, f):
                        b, k = nt // NT_P, nt % NT_P
                        return x1_full[b, f * P:(f + 1) * P, k * P:(k + 1) * P]
                    def src_d(nt, f):
                        b, k = nt // NT_D, nt % NT_D
                        return x1_full[b, f * P:(f + 1) * P, SH_P + k * P:SH_P + (k + 1) * P]
                _kv_phase(nc, sb, pe, wsb[f"wkv_p_{L}"], src_p, kv_p, NP_PAD // P)
                _kv_phase(nc, sb, pe, wsb[f"wkv_d_{L}"], src_d, kv_d, ND_PAD // P)
                _q_phase(nc, sb, pe, wsb[f"wq_p_{L}"], wsb[f"bq_p_{L}"], ones1,
                         x_sh_ap, qt_p, 0, NT_P)
                _q_phase(nc, sb, pe, wsb[f"wq_d_{L}"], wsb[f"bq_d_{L}"], ones1,
                         x_sh_ap, qt_d, SH_P, NT_D)
                arr_cursor = [0, 0]
                # takes: src=patient kv, dst=drug
                _edge_phase(nc, sb, ed, pe_agg, tp, op, plans_t, arr_cursor,
                            edge_idx, edge_off, kv_p, qt_d, iota_f, identity,
                            wsb[f"wout_d_{L}"], wsb[f"bout_d_{L}"], wsb[f"bv_d_{L}"],
                            SKIPC[f"d_{L}"], x_sh_ap, xout_ap, SH_P, NT_D)
                # rev: src=drug kv, dst=patient
                _edge_phase(nc, sb, ed, pe_agg, tp, op, plans_r, arr_cursor,
                            edge_idx, edge_off, kv_d, qt_p, iota_f, identity,
                            wsb[f"wout_p_{L}"], wsb[f"bout_p_{L}"], wsb[f"bv_p_{L}"],
                            SKIPC[f"p_{L}"], x_sh_ap, xout_ap, 0, NT_P)
                if L == 0 and "c" in os.environ.get("KERNEL_SKIP", ""):
                    nc.sync.dma_start(out=x1_full[0], in_=x1_sh)
                elif L == 0:
                    nc.gpsimd.collective_compute(
                        "AllGather", mybir.AluOpType.bypass,
                        replica_groups=[list(range(NCORES))],
                        ins=[x1_sh.opt()], outs=[x1_full.opt()])
    return nc


SKIPC = {}  # filled by kernel() before _build_program
LAST_EXEC_NS = None


def prepare(x_patient, x_drug, ei_takes_src, ei_takes_dst, ei_rev_src, ei_rev_dst, params):
    """Build (nc, in_maps, assemble) without running."""
    global SKIPC
    x_patient = np.asarray(x_patient, np.float32)
    x_drug = np.asarray(x_drug, np.float32)
    W = _fuse_weights(params)
    SKIPC = {f"{t}_{L}": W[f"skipc_{t}_{L}"] for t in ("p", "d") for L in range(2)}

    deg_d = np.bincount(np.asarray(ei_takes_dst), minlength=N_D)
    deg_p = np.bincount(np.asarray(ei_rev_dst), minlength=N_P)
    new_p = _shard_perm(deg_p, N_P, SH_P)
    new_d = _shard_perm(deg_d, N_D, SH_D)

    plans_t, idx_t, off_t = _plan_and_fill(
        np.asarray(ei_takes_src), np.asarray(ei_takes_dst), new_p, new_d, SH_D, NT_D)
    plans_r, idx_r, off_r = _plan_and_fill(
        np.asarray(ei_rev_src), np.asarray(ei_rev_dst), new_d, new_p, SH_P, NT_P)

    edge_idx_c, edge_off_c = [], []
    for c in range(NCORES):
        edge_idx_c.append(np.concatenate(idx_t[c] + idx_r[c]).astype(np.int32))
        edge_off_c.append(np.concatenate(off_t[c] + off_r[c]).astype(np.float32))
    n_edge_idx = len(edge_idx_c[0])
    n_edge_off = len(edge_off_c[0])

    # permuted feature-major inputs
    xp = np.zeros((NP_PAD, IN_P), np.float32)
    xp[new_p] = x_patient
    x0p = np.ascontiguousarray(xp.T)
    xd = np.zeros((ND_PAD, IN_D), np.float32)
    xd[new_d] = x_drug
    x0d = np.ascontiguousarray(xd.T)

    nc = _build_program(plans_t, plans_r, n_edge_idx, n_edge_off)

    in_maps = []
    for c in range(NCORES):
        m = {
            "x0p": x0p, "x0d": x0d,
            "x0p_sh": np.ascontiguousarray(x0p[:, c * SH_P:(c + 1) * SH_P]),
            "x0d_sh": np.ascontiguousarray(x0d[:, c * SH_D:(c + 1) * SH_D]),
            "edge_idx": edge_idx_c[c], "edge_off": edge_off_c[c],
        }
        for name in ("wlin_p", "wlin_d", "blin_p", "blin_d"):
            m[name] = W[name]
        for L in range(2):
            for t in ("p", "d"):
                for stem in ("wkv", "wq", "bq", "wout", "bout", "bv"):
                    name = f"{stem}_{t}_{L}"
                    m[name] = W[name]
        in_maps.append(m)

    nc.compile()

    def assemble(outs):
        full_p = np.concatenate([o[:, :SH_P] for o in outs], axis=1)   # [256, NP_PAD]
        full_d = np.concatenate([o[:, SH_P:] for o in outs], axis=1)   # [256, ND_PAD]
        out_p = full_p.T[new_p].astype(np.float32)
        out_d = full_d.T[new_d].astype(np.float32)
        return (out_p, out_d)

    return nc, in_maps, assemble


def kernel(**inputs):
    nc, in_maps, assemble = prepare(**inputs)
    res = bass_utils.run_bass_kernel_spmd(nc, in_maps, core_ids=list(range(NCORES)))
    return assemble([r["out_sh"] for r in res.results])


if __name__ == "__main__":
    # tiny smoke: random inputs with the real shapes
    rng = np.random.default_rng(0)
    print("smoke run only checks compile+exec plumbing")


# revision 8
# speedup vs baseline: 1.2236x; 1.1692x over previous
"""2-layer HGT (patient/drug) kernel for 8x TRN2 NeuronCores via Bass/Tile.

Strategy:
  - Host: fuse per-head relation matrices R_k/R_v into the K/V projection
    weights, fold p_rel*scale into Q weights, fold sigmoid(skip) into the
    output projection. Permute nodes so destination nodes are degree-sorted
    and dealt round-robin across cores (balanced edges AND nodes); each core
    owns a contiguous shard of permuted node ids.
  - Device (SPMD, one program): feature-major dense projections (TensorE),
    per-edge gather of fused KV rows + Q rows (indirect DMA), logits by
    elementwise mul + segmented reduce (DVE), exp (ACT, no max-subtraction
    needed: softmax is shift-invariant and logits are O(1) here), and
    segment-sum via one-hot selection matmuls accumulating in PSUM.
    One AllGather of node features between the two layers.
"""

import os
import numpy as np

import concourse.bacc as bacc
import concourse.bass as bass
import concourse.mybir as mybir
import concourse.tile as tile
from concourse import bass_utils
from concourse.masks import make_identity

FP32 = mybir.dt.float32
I32 = mybir.dt.int32

# ---- problem constants (hardcoded per contract) ----
N_P, N_D = 50000, 5000
IN_P, IN_D = 128, 64
HID, H, D = 256, 4, 64
NCORES = 8
P = 128
SH_P = 6272          # patient shard cols per core (49 tiles of 128)
SH_D = 640           # drug shard cols per core (5 tiles)
NP_PAD = SH_P * NCORES   # 50176
ND_PAD = SH_D * NCORES   # 5120
NT_P = SH_P // P     # 49
NT_D = SH_D // P     # 5
EW = 4               # max 128-edge subgroups per iteration
SH_ALL = SH_P + SH_D  # 6912

KV_W = 2 * HID       # 512 fused (k_rel | v_rel) row width


# ================= host-side preprocessing =================

def _fuse_weights(params):
    """Returns dict of numpy arrays for device consumption."""
    out = {}
    scale = np.float32(1.0 / np.sqrt(D))
    # input projections
    for t, key in (("p", "patient"), ("d", "drug")):
        W = np.asarray(params["lin"][key]["W"], np.float32)
        b = np.asarray(params["lin"][key]["b"], np.float32)
        out[f"wlin_{t}"] = np.ascontiguousarray(W)
        out[f"blin_{t}"] = np.ascontiguousarray(b.reshape(2, 128).T)  # [128,2] col=fo chunk
    et_src = {"p": "patient__takes__drug", "d": "drug__rev_takes__patient"}
    et_dst = {"p": "drug__rev_takes__patient", "d": "patient__takes__drug"}
    for L, lp in enumerate(params["layers"]):
        for t, key in (("p", "patient"), ("d", "drug")):
            W = np.asarray(lp["kqv"][key]["W"], np.float32)   # [256, 768]
            b = np.asarray(lp["kqv"][key]["b"], np.float32)   # [768]
            Wk, Wq, Wv = W[:, :HID], W[:, HID:2 * HID], W[:, 2 * HID:]
            bk, bq, bv = b[:HID], b[HID:2 * HID], b[2 * HID:]
            Rk = np.asarray(lp["k_rel"][et_src[t]], np.float32)  # [H, D, D]
            Rv = np.asarray(lp["v_rel"][et_src[t]], np.float32)
            prel = np.asarray(lp["p_rel"][et_dst[t]], np.float32)  # [H]
            # k_rel fused (bias dropped: per-dst softmax shift invariance)
            Wk4 = Wk.reshape(HID, H, D)
            Wkf = np.einsum("fhd,hde->fhe", Wk4, Rk).reshape(HID, HID)
            Wv4 = Wv.reshape(HID, H, D)
            Wvf = np.einsum("fhd,hde->fhe", Wv4, Rv).reshape(HID, HID)
            bvf = np.einsum("hd,hde->he", bv.reshape(H, D), Rv).reshape(HID)
            # q scaled by p_rel*scale per head
            qs = (prel * scale).repeat(D)  # [256]
            Wqf = Wq * qs[None, :]
            bqf = bq * qs
            sk = 1.0 / (1.0 + np.exp(-np.float64(lp["skip"][key])))
            sk = np.float32(sk)
            Wo = np.asarray(lp["out"][key]["W"], np.float32) * sk
            bo = np.asarray(lp["out"][key]["b"], np.float32) * sk
            wkv = np.concatenate([Wkf, Wvf], axis=1)  # [256,512]
            out[f"wkv_{t}_{L}"] = np.ascontiguousarray(
                wkv.reshape(2, 128, KV_W).transpose(1, 0, 2).reshape(128, 2 * KV_W))
            out[f"wq_{t}_{L}"] = np.ascontiguousarray(
                Wqf.reshape(2, 128, HID).transpose(1, 0, 2).reshape(128, 2 * HID))
            out[f"bq_{t}_{L}"] = np.ascontiguousarray(bqf.reshape(1, HID))  # [1,256]
            out[f"wout_{t}_{L}"] = np.ascontiguousarray(
                Wo.reshape(2, 128, HID).transpose(1, 0, 2).reshape(128, 2 * HID))
            out[f"bout_{t}_{L}"] = np.ascontiguousarray(bo.reshape(2, 128).T)  # [128,2]
            out[f"bv_{t}_{L}"] = np.ascontiguousarray(bvf.reshape(2, 128).T)   # [128,2]
            out[f"skipc_{t}_{L}"] = np.float32(1.0 - sk)  # python-side const
    return out


def _shard_perm(deg, n, shard):
    """Degree-sorted round-robin dealing. Returns new_of_old [n] -> new id."""
    order = np.argsort(-deg, kind="stable")
    idx = np.arange(n, dtype=np.int64)
    new_ids = (idx % NCORES) * shard + idx // NCORES
    new_of_old = np.empty(n, dtype=np.int64)
    new_of_old[order] = new_ids
    return new_of_old


def _plan_and_fill(src_old, dst_old, new_src, new_dst, shard_dst, ntiles):
    """Build per-core sorted edge lists, a shared iteration plan, and filled
    arrays. Returns (plan_iters [ntiles][...], idx_blocks [NCORES] list of
    int32 arrays, off_blocks [NCORES] list of f32 arrays)."""
    s_new = new_src[src_old].astype(np.int64)
    d_new = new_dst[dst_old].astype(np.int64)
    core = d_new // shard_dst
    slot = d_new % shard_dst
    per_core = []
    counts = np.zeros((NCORES, ntiles), dtype=np.int64)
    for c in range(NCORES):
        m = core == c
        sl = slot[m]
        o = np.argsort(sl, kind="stable")
        per_core.append((s_new[m][o].astype(np.int32), sl[o].astype(np.int32)))
        counts[c] = np.bincount(sl // P, minlength=ntiles)
    padded = ((counts.max(axis=0) + P - 1) // P * P).astype(np.int64)
    plans = []
    for k in range(ntiles):
        n128 = int(padded[k]) // P
        iters = []
        while n128 > 0:
            take = min(EW, n128)
            iters.append(take)
            n128 -= take
        plans.append(iters)
    idx_blocks = [[] for _ in range(NCORES)]
    off_blocks = [[] for _ in range(NCORES)]
    for c in range(NCORES):
        srcs, slots_c = per_core[c]
        pos = 0
        for k in range(ntiles):
            cnt = int(counts[c, k])
            n_pad = int(padded[k])
            s_arr = np.zeros(n_pad, np.int32)
            d_arr = np.zeros(n_pad, np.int32)
            o_arr = np.full(n_pad, -1.0, np.float32)
            s_arr[:cnt] = srcs[pos:pos + cnt]
            d_arr[:cnt] = slots_c[pos:pos + cnt]
            o_arr[:cnt] = (slots_c[pos:pos + cnt] - k * P).astype(np.float32)
            pos += cnt
            eoff = 0
            for S in plans[k]:
                blk = slice(eoff, eoff + S * P)
                sb = s_arr[blk].reshape(S, P).T            # [128, S]
                db = d_arr[blk].reshape(S, P).T
                ob = o_arr[blk].reshape(S, P).T
                idx_blk = np.stack([sb, db], axis=2).reshape(P, S * 2)
                idx_blocks[c].append(np.ascontiguousarray(idx_blk).ravel())
                off_blocks[c].append(np.ascontiguousarray(ob).ravel())
                eoff += S * P
    return plans, idx_blocks, off_blocks


# ================= device program =================

def _inproj(nc, sb, pe, wlin, blin, x0_ap, xh_ap, in_dim, ncols):
    """Feature-major input projection: xh[fo, :] = W.T @ x0 + b."""
    col_tiles = []
    j = 0
    while j < ncols:
        w = min(512, ncols - j)
        col_tiles.append((j, w))
        j += w
    for (j, w) in col_tiles:
        rhs = sb.tile([in_dim, 512], FP32, tag="ip_rhs")
        nc.sync.dma_start(out=rhs[:, :w], in_=x0_ap[:, j:j + w])
        for fo in range(2):
            ps = pe.tile([P, 512], FP32, tag="pe")
            nc.tensor.matmul(ps[:, :w], lhsT=wlin[:, fo * P:(fo + 1) * P],
                             rhs=rhs[:, :w], start=True, stop=True)
            ot = sb.tile([P, 512], FP32, tag="ip_out")
            nc.scalar.activation(ot[:, :w], ps[:, :w],
                                 mybir.ActivationFunctionType.Identity,
                                 bias=blin[:, fo:fo + 1])
            nc.sync.dma_start(out=xh_ap[fo * P:(fo + 1) * P, j:j + w], in_=ot[:, :w])


def _kv_phase(nc, sb, pe, wkv, src_chunks, kv_ap, ntiles):
    """kv[node, :] = x.T @ Wkv (no bias). src_chunks(nt, f) -> AP [128,128]."""
    for nt in range(ntiles):
        ps = pe.tile([P, KV_W], FP32, tag="pe")
        for f in range(2):
            lx = sb.tile([P, P], FP32, tag="kv_lx")
            nc.sync.dma_start(out=lx[:], in_=src_chunks(nt, f))
            nc.tensor.matmul(ps[:], lhsT=lx[:], rhs=wkv[:, f * KV_W:(f + 1) * KV_W],
                             start=(f == 0), stop=(f == 1))
        ot = sb.tile([P, KV_W], FP32, tag="kv_out")
        nc.scalar.copy(ot[:], ps[:])
        nc.sync.dma_start(out=kv_ap[nt * P:(nt + 1) * P, :], in_=ot[:])


def _q_phase(nc, sb, pe, wq, bq_row, ones1, x_sh_ap, q_ap, base, ntiles):
    for nt in range(ntiles):
        ps = pe.tile([P, HID], FP32, tag="pe")
        for f in range(2):
            lx = sb.tile([P, P], FP32, tag="q_lx")
            nc.sync.dma_start(out=lx[:], in_=x_sh_ap[f * P:(f + 1) * P,
                                                     base + nt * P: base + (nt + 1) * P])
            nc.tensor.matmul(ps[:], lhsT=lx[:], rhs=wq[:, f * HID:(f + 1) * HID],
                             start=(f == 0), stop=False)
        nc.tensor.matmul(ps[:], lhsT=ones1[:, :P], rhs=bq_row[:, :],
                         start=False, stop=True)
        ot = sb.tile([P, HID], FP32, tag="q_out")
        nc.scalar.copy(ot[:], ps[:])
        nc.sync.dma_start(out=q_ap[nt * P:(nt + 1) * P, :], in_=ot[:])


def _edge_phase(nc, sb, ed, pe_agg, tp, op, plans, arr_cursor,
                edge_idx_ap, edge_off_ap, kv_ap, q_ap, iota_f, identity,
                wout, bout, bv, skipc, x_sh_ap, xout_ap, col_base,
                ntiles):
    """Edge phase + epilogue + out-proj for one (layer, edge type)."""
    for k in range(ntiles):
        iters = plans[k]
        if "e" in os.environ.get("KERNEL_SKIP", ""):
            iters = []
        nsub_total = sum(iters)
        agg = pe_agg.tile([P, 260], FP32, tag="agg")
        sub_i = 0
        for S in iters:
            io, oo = arr_cursor[0], arr_cursor[1]
            arr_cursor[0] += P * S * 2
            arr_cursor[1] += P * S
            idx = ed.tile([P, EW * 2], I32, tag="idx")
            nc.sync.dma_start(out=idx[:, :S * 2],
                              in_=edge_idx_ap[io:io + P * S * 2].rearrange("(p x) -> p x", p=P))
            dstf = ed.tile([P, EW], FP32, tag="dstf")
            nc.sync.dma_start(out=dstf[:, :S],
                              in_=edge_off_ap[oo:oo + P * S].rearrange("(p x) -> p x", p=P))
            kvg = ed.tile([P, EW * KV_W], FP32, tag="kvg")
            qg = ed.tile([P, EW * HID], FP32, tag="qg")
            for s in range(S):
                nc.gpsimd.indirect_dma_start(
                    out=kvg[:, s * KV_W:(s + 1) * KV_W], out_offset=None,
                    in_=kv_ap,
                    in_offset=bass.IndirectOffsetOnAxis(ap=idx[:, 2 * s:2 * s + 1], axis=0))
                if "g" in os.environ.get("KERNEL_SKIP", ""):
                    pass
                else:
                    nc.gpsimd.indirect_dma_start(
                        out=qg[:, s * HID:(s + 1) * HID], out_offset=None,
                        in_=q_ap,
                        in_offset=bass.IndirectOffsetOnAxis(ap=idx[:, 2 * s + 1:2 * s + 2], axis=0))
            prod = ed.tile([P, EW * HID], FP32, tag="prod")
            kvg_k = kvg[:, :S * KV_W].rearrange("p (s x) -> p s x", x=KV_W)[:, :, 0:HID]
            qg_in = qg[:, :S * HID] if "g" not in os.environ.get("KERNEL_SKIP", "") else \
                kvg[:, :S * KV_W].rearrange("p (s x) -> p s x", x=KV_W)[:, :, HID:2 * HID]
            nc.vector.tensor_tensor(out=prod[:, :S * HID], in0=kvg_k,
                                    in1=qg_in, op=mybir.AluOpType.mult)
            lg = ed.tile([P, EW * H], FP32, tag="lg")
            nc.vector.tensor_reduce(
                out=lg[:, :S * H],
                in_=prod[:, :S * HID].rearrange("p (x d) -> p x d", d=D),
                axis=mybir.AxisListType.X, op=mybir.AluOpType.add)
            ex = ed.tile([P, EW * H], FP32, tag="ex")
            nc.scalar.activation(ex[:, :S * H], lg[:, :S * H],
                                 mybir.ActivationFunctionType.Exp)
            rhs = ed.tile([P, EW * 260], FP32, tag="rhs")
            for s in range(S):
                for h in range(H):
                    nc.vector.tensor_scalar(
                        out=rhs[:, s * 260 + h * D:s * 260 + (h + 1) * D],
                        in0=kvg[:, s * KV_W + HID + h * D:s * KV_W + HID + (h + 1) * D],
                        scalar1=ex[:, s * H + h:s * H + h + 1], scalar2=None,
                        op0=mybir.AluOpType.mult)
            nc.vector.tensor_copy(
                out=rhs[:, :S * 260].rearrange("p (s x) -> p s x", x=260)[:, :, 256:260],
                in_=ex[:, :S * H].rearrange("p (s h) -> p s h", h=H))
            oneh = ed.tile([P, EW * P], FP32, tag="oneh")
            for s in range(S):
                nc.vector.tensor_tensor(
                    out=oneh[:, s * P:(s + 1) * P],
                    in0=dstf[:, s:s + 1].to_broadcast([P, P]),
                    in1=iota_f[:, :], op=mybir.AluOpType.is_equal)
            for s in range(S):
                nc.tensor.matmul(agg[:], lhsT=oneh[:, s * P:(s + 1) * P],
                                 rhs=rhs[:, s * 260:(s + 1) * 260],
                                 start=(sub_i == 0), stop=(sub_i == nsub_total - 1))
                sub_i += 1
        # ---- epilogue: normalize, gelu, out-proj, skip ----
        aggsb = sb.tile([P, HID], FP32, tag="aggsb")
        if nsub_total == 0:
            nc.vector.memset(aggsb[:], 0.0)
        else:
            sden = sb.tile([P, H], FP32, tag="sden")
            nc.vector.tensor_scalar(out=sden[:], in0=agg[:, 256:260],
                                    scalar1=1e-16, scalar2=None,
                                    op0=mybir.AluOpType.add)
            recip = sb.tile([P, H], FP32, tag="recip")
            nc.vector.reciprocal(recip[:], sden[:])
            for h in range(H):
                nc.scalar.activation(aggsb[:, h * D:(h + 1) * D],
                                     agg[:, h * D:(h + 1) * D],
                                     mybir.ActivationFunctionType.Copy,
                                     scale=recip[:, h:h + 1])
        gelu_fm = sb.tile([P, HID], FP32, tag="gelu_fm")
        for c in range(2):
            tpp = tp.tile([P, P], FP32, tag="tp")
            nc.tensor.transpose(tpp[:], aggsb[:, c * P:(c + 1) * P], identity[:])
            nc.scalar.activation(gelu_fm[:, c * P:(c + 1) * P], tpp[:],
                                 mybir.ActivationFunctionType.Gelu,
                                 bias=bv[:, c:c + 1])
        for fo in range(2):
            pso = op.tile([P, P], FP32, tag="op")
            for c in range(2):
                nc.tensor.matmul(pso[:], lhsT=wout[:, c * HID + fo * P:c * HID + (fo + 1) * P],
                                 rhs=gelu_fm[:, c * P:(c + 1) * P],
                                 start=(c == 0), stop=(c == 1))
            oo_t = sb.tile([P, P], FP32, tag="oo")
            nc.scalar.activation(oo_t[:], pso[:],
                                 mybir.ActivationFunctionType.Identity,
                                 bias=bout[:, fo:fo + 1])
            xs = sb.tile([P, P], FP32, tag="xs")
            nc.sync.dma_start(out=xs[:], in_=x_sh_ap[fo * P:(fo + 1) * P,
                                                     col_base + k * P: col_base + (k + 1) * P])
            xs2 = sb.tile([P, P], FP32, tag="xs2")
            nc.vector.tensor_scalar(out=xs2[:], in0=xs[:], scalar1=float(skipc),
                                    scalar2=None, op0=mybir.AluOpType.mult)
            nc.vector.tensor_tensor(out=oo_t[:], in0=oo_t[:], in1=xs2[:],
                                    op=mybir.AluOpType.add)
            nc.sync.dma_start(out=xout_ap[fo * P:(fo + 1) * P,
                                          col_base + k * P: col_base + (k + 1) * P],
                              in_=oo_t[:])


def _build_program(plans_t, plans_r, n_edge_idx, n_edge_off, trace_scopes=False):
    nc = bacc.Bacc("TRN2", target_bir_lowering=False, debug=False,
                   enable_asserts=False, num_devices=NCORES)
    dt = FP32
    # ---- I/O ----
    x0p = nc.dram_tensor("x0p", [IN_P, NP_PAD], dt, kind="ExternalInput").ap()
    x0d = nc.dram_tensor("x0d", [IN_D, ND_PAD], dt, kind="ExternalInput").ap()
    x0p_sh = nc.dram_tensor("x0p_sh", [IN_P, SH_P], dt, kind="ExternalInput").ap()
    x0d_sh = nc.dram_tensor("x0d_sh", [IN_D, SH_D], dt, kind="ExternalInput").ap()
    edge_idx = nc.dram_tensor("edge_idx", [n_edge_idx], I32, kind="ExternalInput").ap()
    edge_off = nc.dram_tensor("edge_off", [n_edge_off], dt, kind="ExternalInput").ap()
    win = {}
    for t, ind in (("p", IN_P), ("d", IN_D)):
        win[f"wlin_{t}"] = nc.dram_tensor(f"wlin_{t}", [ind, HID], dt, kind="ExternalInput").ap()
        win[f"blin_{t}"] = nc.dram_tensor(f"blin_{t}", [P, 2], dt, kind="ExternalInput").ap()
    for L in range(2):
        for t in ("p", "d"):
            win[f"wkv_{t}_{L}"] = nc.dram_tensor(f"wkv_{t}_{L}", [P, 2 * KV_W], dt, kind="ExternalInput").ap()
            win[f"wq_{t}_{L}"] = nc.dram_tensor(f"wq_{t}_{L}", [P, 2 * HID], dt, kind="ExternalInput").ap()
            win[f"bq_{t}_{L}"] = nc.dram_tensor(f"bq_{t}_{L}", [1, HID], dt, kind="ExternalInput").ap()
            win[f"wout_{t}_{L}"] = nc.dram_tensor(f"wout_{t}_{L}", [P, 2 * HID], dt, kind="ExternalInput").ap()
            win[f"bout_{t}_{L}"] = nc.dram_tensor(f"bout_{t}_{L}", [P, 2], dt, kind="ExternalInput").ap()
            win[f"bv_{t}_{L}"] = nc.dram_tensor(f"bv_{t}_{L}", [P, 2], dt, kind="ExternalInput").ap()
    out_sh = nc.dram_tensor("out_sh", [HID, SH_ALL], dt, kind="ExternalOutput").ap()
    # ---- DRAM scratch ----
    xh_p = nc.dram_tensor("xh_p", [HID, NP_PAD], dt, kind="Internal").ap()
    xh_d = nc.dram_tensor("xh_d", [HID, ND_PAD], dt, kind="Internal").ap()
    xh_sh = nc.dram_tensor("xh_sh", [HID, SH_ALL], dt, kind="Internal").ap()
    kv_p = nc.dram_tensor("kv_p", [NP_PAD, KV_W], dt, kind="Internal").ap()
    kv_d = nc.dram_tensor("kv_d", [ND_PAD, KV_W], dt, kind="Internal").ap()
    qt_p = nc.dram_tensor("qt_p", [SH_P, HID], dt, kind="Internal").ap()
    qt_d = nc.dram_tensor("qt_d", [SH_D, HID], dt, kind="Internal").ap()
    x1_sh = nc.dram_tensor("x1_sh", [HID, SH_ALL], dt, kind="Internal").ap()
    x1_full = nc.dram_tensor("x1_full", [NCORES, HID, SH_ALL], dt, kind="Internal",
                             addr_space="Shared").ap()

    with tile.TileContext(nc) as tc:
        with (
            tc.tile_pool(name="const", bufs=1) as cp,
            tc.tile_pool(name="sb", bufs=3) as sb,
            tc.tile_pool(name="ed", bufs=3) as ed,
            tc.tile_pool(name="pe", bufs=2, space="PSUM") as pe,
            tc.tile_pool(name="agg", bufs=2, space="PSUM") as pe_agg,
            tc.tile_pool(name="tp", bufs=2, space="PSUM") as tp,
            tc.tile_pool(name="op", bufs=2, space="PSUM") as op,
        ):
            # consts
            identity = cp.tile([P, P], FP32)
            make_identity(nc, identity[:])
            iota_i = cp.tile([P, P], I32)
            nc.gpsimd.iota(iota_i[:], pattern=[[1, P]], base=0, channel_multiplier=0)
            iota_f = cp.tile([P, P], FP32)
            nc.vector.tensor_copy(iota_f[:], iota_i[:])
            ones1 = cp.tile([1, P], FP32)
            nc.vector.memset(ones1[:], 1.0)
            # weights to SBUF
            wsb = {}
            for name, ap in win.items():
                t_ = cp.tile(list(ap.shape), FP32, tag=name)
                nc.sync.dma_start(out=t_[:], in_=ap)
                wsb[name] = t_

            # ---- Phase A: input projections ----
            if "i" not in os.environ.get("KERNEL_SKIP", ""):
                _inproj(nc, sb, pe, wsb["wlin_p"], wsb["blin_p"], x0p, xh_p, IN_P, NP_PAD)
                _inproj(nc, sb, pe, wsb["wlin_d"], wsb["blin_d"], x0d, xh_d, IN_D, ND_PAD)
            _inproj(nc, sb, pe, wsb["wlin_p"], wsb["blin_p"], x0p_sh,
                    xh_sh[:, 0:SH_P], IN_P, SH_P)
            _inproj(nc, sb, pe, wsb["wlin_d"], wsb["blin_d"], x0d_sh,
                    xh_sh[:, SH_P:SH_ALL], IN_D, SH_D)

            for L in range(2):
                x_sh_ap = xh_sh if L == 0 else x1_sh
                xout_ap = x1_sh if L == 0 else out_sh
                if L == 0:
                    src_p = lambda nt, f: xh_p[f * P:(f + 1) * P, nt * P:(nt + 1) * P]
                    src_d = lambda nt, f: xh_d[f * P:(f + 1) * P, nt * P:(nt + 1) * P]
                else:
                    def src_p(nt, f):
                        b, k = nt // NT_P, nt % NT_P
                        return x1_full[b, f * P:(f + 1) * P, k * P:(k + 1) * P]
                    def src_d(nt, f):
                        b, k = nt // NT_D, nt % NT_D
                        return x1_full[b, f * P:(f + 1) * P, SH_P + k * P:SH_P + (k + 1) * P]
                if "k" not in os.environ.get("KERNEL_SKIP", ""):
                    _kv_phase(nc, sb, pe, wsb[f"wkv_p_{L}"], src_p, kv_p, NP_PAD // P)
                    _kv_phase(nc, sb, pe, wsb[f"wkv_d_{L}"], src_d, kv_d, ND_PAD // P)
                if "q" not in os.environ.get("KERNEL_SKIP", ""):
                    _q_phase(nc, sb, pe, wsb[f"wq_p_{L}"], wsb[f"bq_p_{L}"], ones1,
                             x_sh_ap, qt_p, 0, NT_P)
                    _q_phase(nc, sb, pe, wsb[f"wq_d_{L}"], wsb[f"bq_d_{L}"], ones1,
                             x_sh_ap, qt_d, SH_P, NT_D)
                arr_cursor = [0, 0]
                # takes: src=patient kv, dst=drug
                _edge_phase(nc, sb, ed, pe_agg, tp, op, plans_t, arr_cursor,
                            edge_idx, edge_off, kv_p, qt_d, iota_f, identity,
                            wsb[f"wout_d_{L}"], wsb[f"bout_d_{L}"], wsb[f"bv_d_{L}"],
                            SKIPC[f"d_{L}"], x_sh_ap, xout_ap, SH_P, NT_D)
                # rev: src=drug kv, dst=patient
                _edge_phase(nc, sb, ed, pe_agg, tp, op, plans_r, arr_cursor,
                            edge_idx, edge_off, kv_d, qt_p, iota_f, identity,
                            wsb[f"wout_p_{L}"], wsb[f"bout_p_{L}"], wsb[f"bv_p_{L}"],
                            SKIPC[f"p_{L}"], x_sh_ap, xout_ap, 0, NT_P)
                if L == 0 and "c" in os.environ.get("KERNEL_SKIP", ""):
                    nc.sync.dma_start(out=x1_full[0], in_=x1_sh)
                elif L == 0:
                    nc.gpsimd.collective_compute(
                        "AllGather", mybir.AluOpType.bypass,
                        replica_groups=[list(range(NCORES))],
                        ins=[x1_sh.opt()], outs=[x1_full.opt()])
    return nc


SKIPC = {}  # filled by kernel() before _build_program
LAST_EXEC_NS = None


def prepare(x_patient, x_drug, ei_takes_src, ei_takes_dst, ei_rev_src, ei_rev_dst, params):
    """Build (nc, in_maps, assemble) without running."""
    global SKIPC
    x_patient = np.asarray(x_patient, np.float32)
    x_drug = np.asarray(x_drug, np.float32)
    W = _fuse_weights(params)
    SKIPC = {f"{t}_{L}": W[f"skipc_{t}_{L}"] for t in ("p", "d") for L in range(2)}

    deg_d = np.bincount(np.asarray(ei_takes_dst), minlength=N_D)
    deg_p = np.bincount(np.asarray(ei_rev_dst), minlength=N_P)
    new_p = _shard_perm(deg_p, N_P, SH_P)
    new_d = _shard_perm(deg_d, N_D, SH_D)

    plans_t, idx_t, off_t = _plan_and_fill(
        np.asarray(ei_takes_src), np.asarray(ei_takes_dst), new_p, new_d, SH_D, NT_D)
    plans_r, idx_r, off_r = _plan_and_fill(
        np.asarray(ei_rev_src), np.asarray(ei_rev_dst), new_d, new_p, SH_P, NT_P)

    edge_idx_c, edge_off_c = [], []
    for c in range(NCORES):
        edge_idx_c.append(np.concatenate(idx_t[c] + idx_r[c]).astype(np.int32))
        edge_off_c.append(np.concatenate(off_t[c] + off_r[c]).astype(np.float32))
    n_edge_idx = len(edge_idx_c[0])
    n_edge_off = len(edge_off_c[0])

    # permuted feature-major inputs
    xp = np.zeros((NP_PAD, IN_P), np.float32)
    xp[new_p] = x_patient
    x0p = np.ascontiguousarray(xp.T)
    xd = np.zeros((ND_PAD, IN_D), np.float32)
    xd[new_d] = x_drug
    x0d = np.ascontiguousarray(xd.T)

    nc = _build_program(plans_t, plans_r, n_edge_idx, n_edge_off)

    in_maps = []
    for c in range(NCORES):
        m = {
            "x0p": x0p, "x0d": x0d,
            "x0p_sh": np.ascontiguousarray(x0p[:, c * SH_P:(c + 1) * SH_P]),
            "x0d_sh": np.ascontiguousarray(x0d[:, c * SH_D:(c + 1) * SH_D]),
            "edge_idx": edge_idx_c[c], "edge_off": edge_off_c[c],
        }
        for name in ("wlin_p", "wlin_d", "blin_p", "blin_d"):
            m[name] = W[name]
        for L in range(2):
            for t in ("p", "d"):
                for stem in ("wkv", "wq", "bq", "wout", "bout", "bv"):
                    name = f"{stem}_{t}_{L}"
                    m[name] = W[name]
        in_maps.append(m)

    nc.compile()

    def assemble(outs):
        full_p = np.concatenate([o[:, :SH_P] for o in outs], axis=1)   # [256, NP_PAD]
        full_d = np.concatenate([o[:, SH_P:] for o in outs], axis=1)   # [256, ND_PAD]
        out_p = full_p.T[new_p].astype(np.float32)
        out_d = full_d.T[new_d].astype(np.float32)
        return (out_p, out_d)

    return nc, in_maps, assemble


def kernel(**inputs):
    nc, in_maps, assemble = prepare(**inputs)
    res = bass_utils.run_bass_kernel_spmd(nc, in_maps, core_ids=list(range(NCORES)))
    return assemble([r["out_sh"] for r in res.results])


if __name__ == "__main__":
    # tiny smoke: random inputs with the real shapes
    rng = np.random.default_rng(0)
    print("smoke run only checks compile+exec plumbing")
